# revision 5
# baseline (speedup 1.0000x reference)
"""GCN block (GCNConv + BN(eval) + ReLU) on 8 Trainium2 NeuronCores.

Strategy (fully data-parallel, no collectives):
  out = relu(BN(D^{-1/2}(A+I)D^{-1/2} (x W) + b))
      = relu(dis_dst * ((sum_{e->dst} xs[src] + xs[dst]) @ W') + b')
  where xs = x * dis (dis = deg^{-1/2}), W' = W * s, b' = b*s + t (BN folded).

  Nodes are sharded across 8 cores by destination block (degree-balanced
  snake deal).  Per destination, the self-loop row plus the edge source
  rows are PAIRED; pair members are laid out as two parallel HBM streams
  (bf16 xs rows, duplicated per item).  Stream A arrives via the hardware
  DGE channel as a plain write; stream B arrives via the gpsimd software
  DGE channel with accum_op=add, so the DMA engines themselves perform the
  pair reduction into SBUF — halving both the DMA bytes per channel and
  the tensor-engine selection work.  One-hot selection matrices (built on
  DVE from per-slot dst ids) reduce pair-slots into [feat, dst] PSUM; a
  512x512 transform GEMM, K=1 bias matmul (bias pre-scaled by 1/dis), and
  a fused dis*ReLU activation produce bf16 output.
"""

import sys

if "/opt/trn_rl_repo" not in sys.path:
    sys.path.insert(0, "/opt/trn_rl_repo")

import math

import ml_dtypes
import numpy as np

BF16 = ml_dtypes.bfloat16

N_CORES = 8
P = 128
BN_EPS = 1e-5
TB = 7  # dst tiles per batch


def _prep(x, edge_index, W, b, gamma, beta, running_mean, running_var):
    """Host-side preprocessing: sharding, pair layout, BN folding."""
    N, F = x.shape
    F_OUT = W.shape[1]
    KC = F // P
    assert N % N_CORES == 0
    NB = N // N_CORES
    T = math.ceil(NB / P)  # dst tiles per core

    src = np.asarray(edge_index[0], dtype=np.int64)
    dst = np.asarray(edge_index[1], dtype=np.int64)

    deg = 1.0 + np.bincount(dst, minlength=N).astype(np.float64)
    dis = (1.0 / np.sqrt(deg)).astype(np.float32)

    xs = (np.asarray(x, np.float32) * dis[:, None]).astype(BF16)

    # BN folding
    s = (np.asarray(gamma, np.float32)
         / np.sqrt(np.asarray(running_var, np.float32) + BN_EPS))
    t = np.asarray(beta, np.float32) - np.asarray(running_mean, np.float32) * s
    Wp = (np.asarray(W, np.float32) * s[None, :]).astype(BF16)
    bp = (np.asarray(b, np.float32) * s + t).astype(np.float32)
    wp = np.ascontiguousarray(Wp.reshape(KC, P, F_OUT).transpose(1, 0, 2))

    # ---- degree-balanced node -> (core, tile, slot) assignment (snake deal)
    NBINS = N_CORES * T
    order = np.argsort(-(deg - 1.0), kind="stable")
    assign = np.empty(N, np.int64)   # node -> bin
    slot_of = np.empty(N, np.int64)  # node -> slot within bin
    pos = 0
    rnd = 0
    while pos < N:
        chunk = order[pos:pos + NBINS]
        if rnd % 2 == 0:
            bins = np.arange(len(chunk))
        else:
            bins = NBINS - 1 - np.arange(len(chunk))
        assign[chunk] = bins
        slot_of[chunk] = rnd
        pos += NBINS
        rnd += 1
    assert rnd <= P, f"too many slot rounds {rnd}"
    core_of_bin = assign % N_CORES
    tile_of_bin = assign // N_CORES

    # node_map[k][t, p] = original node id (or -1)
    node_map = np.full((N_CORES, T, P), -1, dtype=np.int64)
    node_map[core_of_bin, tile_of_bin, slot_of] = np.arange(N)

    e_core = core_of_bin[dst]
    e_tile = tile_of_bin[dst]
    e_slot = slot_of[dst]

    # ---- pass 1: per (core, tile) pair lists (self + edges, chunked by 2)
    per_core = []
    n_pairs = np.zeros((N_CORES, T), dtype=np.int64)
    for k in range(N_CORES):
        m = e_core == k
        s_k = src[m]
        key = e_tile[m] * P + e_slot[m]
        o = np.argsort(key, kind="stable")
        s_k = s_k[o]
        key = key[o]
        # per (tile,slot) degree, aligned to the [T, P] grid
        degg = np.bincount(key, minlength=T * P).reshape(T, P)
        nm = node_map[k]
        valid = nm >= 0
        items_cnt = np.where(valid, 1 + degg, 0)      # self + edges
        cnt_pe = items_cnt + (items_cnt & 1)          # padded to even
        pairs_ts = cnt_pe // 2                        # pairs per (t, p)
        n_pairs[k] = pairs_ts.sum(axis=1)
        per_core.append((s_k, degg, items_cnt, pairs_ts))

    # batch-uniform group counts: every tile in a batch shares NG
    batches = [(t0, min(t0 + TB, T)) for t0 in range(0, T, TB)]
    NGt = np.ceil(n_pairs.max(axis=0) / P).astype(np.int64)
    NGt = np.maximum(NGt, 1)
    NG_b = {}
    NG_t = np.zeros(T, np.int64)
    for t0, t1 in batches:
        g = int(NGt[t0:t1].max())
        NG_b[t0] = g
        NG_t[t0:t1] = g
    S_t = NG_t * P
    off_t = np.concatenate([[0], np.cumsum(S_t)])
    TOT = int(off_t[-1])
    NGTOT = TOT // P

    # ---- pass 2: per-core arrays
    in_maps = []
    for k in range(N_CORES):
        s_k, degg, items_cnt, pairs_ts = per_core[k]
        nm = node_map[k]
        valid = nm >= 0
        nm_safe = np.where(valid, nm, 0)

        # Build per-tile item arrays: items of dst p = [own, src...src],
        # padded to even with -1.
        gA = np.zeros((NGTOT, P, F), dtype=BF16)    # [g, p, F]
        gB = np.zeros((NGTOT, P, F), dtype=BF16)
        dstl = np.full((NGTOT, P), -1.0, dtype=np.float32)

        ecnt = degg.reshape(-1)                      # [T*P] edge counts
        e_ofs = np.concatenate([[0], np.cumsum(ecnt)])  # into s_k
        cnt_pe = (items_cnt + (items_cnt & 1)).reshape(-1)
        i_ofs = np.concatenate([[0], np.cumsum(cnt_pe)])
        total_items = int(i_ofs[-1])
        items = np.full(total_items, -1, dtype=np.int64)
        # self items at segment starts (only where valid)
        vmask = valid.reshape(-1)
        items[i_ofs[:-1][vmask]] = nm_safe.reshape(-1)[vmask]
        # edges at ofs + 1 + rank
        rank = np.arange(len(s_k)) - np.repeat(e_ofs[:-1], ecnt)
        gidx = np.repeat(np.arange(T * P), ecnt)
        items[i_ofs[:-1][gidx] + 1 + rank] = s_k

        itemsA = items[0::2]
        itemsB = items[1::2]
        pair_dst = np.repeat(np.arange(T * P) % P, cnt_pe // 2)
        pair_tile = np.repeat(np.arange(T * P) // P, cnt_pe // 2)

        # slot within tile, then grid cell (p_cell, gp)
        ppt = pairs_ts.sum(axis=1)                   # pairs per tile
        p_ofs = np.concatenate([[0], np.cumsum(ppt)])
        srank = np.arange(len(pair_dst)) - np.repeat(p_ofs[:-1], ppt)
        p_cell = srank % P
        gp = (off_t[:-1][pair_tile] // P) + srank // P

        gA[gp, p_cell] = xs[itemsA]
        mB = itemsB >= 0
        gB[gp[mB], p_cell[mB]] = xs[itemsB[mB]]
        dstl[gp, p_cell] = pair_dst.astype(np.float32)

        gA = np.ascontiguousarray(gA.transpose(1, 0, 2))   # [P, NGTOT, F]
        gB = np.ascontiguousarray(gB.transpose(1, 0, 2))
        dstl = np.ascontiguousarray(dstl.T)                 # [P, NGTOT]

        iota = np.ascontiguousarray(np.broadcast_to(
            np.arange(P, dtype=np.float32), (P, P)).astype(BF16))

        dis_tp = np.where(valid, dis[nm_safe], 1.0).astype(np.float32)  # [T,P]
        dis_t = np.ascontiguousarray(dis_tp.T)  # [128, T]
        invdis = np.zeros((1, T * P), dtype=BF16)
        invdis[0, :] = np.where(valid, 1.0 / np.maximum(dis_tp, 1e-9), 0.0
                                ).reshape(-1).astype(BF16)
        in_maps.append({
            "iota": iota,
            "gA": gA,
            "gB": gB,
            "dstl": dstl,
            "dis_t": dis_t,
            "invdis": invdis,
            "wp": wp,
            "bp": bp.reshape(1, F_OUT).astype(BF16),
        })

    meta = {
        "N": N, "F": F, "F_OUT": F_OUT, "KC": KC, "NB": NB, "T": T,
        "TOT": TOT, "NGTOT": NGTOT,
        "S_t": S_t.tolist(), "off_t": off_t.tolist(), "NG_t": NG_t.tolist(),
        "node_map": node_map,
    }
    return meta, in_maps


def _build_program(meta):
    """Emit the Bass/Tile program (shared by all cores)."""
    import concourse.bacc as bacc
    import concourse.mybir as mybir
    import concourse.tile as tile

    F, F_OUT, KC = meta["F"], meta["F_OUT"], meta["KC"]
    T, NGTOT = meta["T"], meta["NGTOT"]
    off_t, NG_t = meta["off_t"], meta["NG_t"]

    dt = mybir.dt
    nc = bacc.Bacc("TRN2", target_bir_lowering=False, debug=False,
                   enable_asserts=False, num_devices=N_CORES,
                   num_swdge_queues=4)

    gA = nc.dram_tensor("gA", [P, NGTOT, F], dt.bfloat16, kind="ExternalInput").ap()
    gB = nc.dram_tensor("gB", [P, NGTOT, F], dt.bfloat16, kind="ExternalInput").ap()
    dstl = nc.dram_tensor("dstl", [P, NGTOT], dt.float32, kind="ExternalInput").ap()
    iota = nc.dram_tensor("iota", [P, P], dt.bfloat16, kind="ExternalInput").ap()
    dis_t = nc.dram_tensor("dis_t", [P, T], dt.float32, kind="ExternalInput").ap()
    invdis = nc.dram_tensor("invdis", [1, T * P], dt.bfloat16, kind="ExternalInput").ap()
    wp = nc.dram_tensor("wp", [P, KC, F_OUT], dt.bfloat16, kind="ExternalInput").ap()
    bp = nc.dram_tensor("bp", [1, F_OUT], dt.bfloat16, kind="ExternalInput").ap()
    out = nc.dram_tensor("out", [P, T, F_OUT], dt.bfloat16, kind="ExternalOutput").ap()

    batches = [(t0, min(t0 + TB, T)) for t0 in range(0, T, TB)]
    max_bw = max(off_t[t1] // P - off_t[t0] // P for t0, t1 in batches)

    with tile.TileContext(nc) as tc:
        with (
            tc.tile_pool(name="const", bufs=1) as cpool,
            tc.tile_pool(name="gbuf", bufs=3) as gpool,
            tc.tile_pool(name="selb", bufs=2) as selpool,
            tc.tile_pool(name="small", bufs=2) as spool,
            tc.tile_pool(name="aggT", bufs=3) as aggpool,
            tc.tile_pool(name="outsb", bufs=2) as opool,
            tc.tile_pool(name="psA", bufs=2, space="PSUM") as psA,
            tc.tile_pool(name="psB", bufs=2, space="PSUM") as psB,
        ):
            # resident constants
            iota_sb = cpool.tile([P, P], dt.bfloat16, tag="iota")
            nc.sync.dma_start(iota_sb[:], iota[:])
            dis_sb = cpool.tile([P, T], dt.float32, tag="dis")
            nc.sync.dma_start(dis_sb[:], dis_t[:])
            invdis_sb = cpool.tile([1, T * P], dt.bfloat16, tag="invdis")
            nc.sync.dma_start(invdis_sb[:], invdis[:])
            wp_sb = cpool.tile([P, KC, F_OUT], dt.bfloat16, tag="wp")
            nc.sync.dma_start(wp_sb[:], wp[:])
            bp_sb = cpool.tile([1, F_OUT], dt.bfloat16, tag="bp")
            nc.sync.dma_start(bp_sb[:], bp[:])

            for t0, t1 in batches:
                nb_t = t1 - t0
                go0, go1 = off_t[t0] // P, off_t[t1] // P
                bw = go1 - go0

                dstl_sb = spool.tile([P, max_bw], dt.float32, tag="dstl")
                nc.sync.dma_start(dstl_sb[:, :bw], dstl[:, go0:go1])
                # pair reduction in DMA: A = plain write (HW DGE),
                # B = accumulate (SW DGE on gpsimd)
                g2_sb = gpool.tile([P, max_bw, F], dt.bfloat16, tag="g2")
                nc.sync.dma_start(g2_sb[:, :bw, :], gA[:, go0:go1, :])
                # accum DMA is only correct for calls <= 512KB (4 groups)
                for g0 in range(0, bw, 4):
                    g1 = min(g0 + 4, bw)
                    nc.gpsimd.dma_start(g2_sb[:, g0:g1, :],
                                        gB[:, go0 + g0:go0 + g1, :],
                                        accum_op=mybir.AluOpType.add)

                sel_sb = selpool.tile([P, max_bw, P], dt.bfloat16, tag="sel")
                out_blk = opool.tile([P, TB, F_OUT], dt.bfloat16, tag="out_sb")

                for t in range(t0, t1):
                    ng = NG_t[t]
                    lg = off_t[t] // P - go0

                    # sel[p, c] = (iota[p,c] == dstl[p,g])
                    for g in range(ng):
                        nc.vector.tensor_scalar(
                            out=sel_sb[:, lg + g, :], in0=iota_sb[:],
                            scalar1=dstl_sb[:, lg + g:lg + g + 1],
                            scalar2=None,
                            op0=mybir.AluOpType.is_equal)

                    # selection matmuls: aggT[fchunk, dst] += G2_chunk^T @ sel
                    aggT_ps = psA.tile([P, F], dt.float32, tag="aggT_ps")
                    for g in range(ng):
                        for c in range(KC):
                            nc.tensor.matmul(
                                aggT_ps[:, c * P:(c + 1) * P],
                                lhsT=g2_sb[:, lg + g, c * P:(c + 1) * P],
                                rhs=sel_sb[:, lg + g, :],
                                start=(g == 0 and c == 0),
                                stop=(g == ng - 1 and c == KC - 1),
                                skip_group_check=True,
                            )

                    aggT_sb = aggpool.tile([P, F], dt.bfloat16, tag="aggT_sb")
                    nc.vector.tensor_copy(aggT_sb[:], aggT_ps[:])

                    # transform GEMM + K=1 bias row (bias pre-scaled by 1/dis)
                    out_ps = psB.tile([P, F_OUT], dt.float32, tag="out_ps")
                    for c in range(KC):
                        nc.tensor.matmul(
                            out_ps[:],
                            lhsT=aggT_sb[:, c * P:(c + 1) * P],
                            rhs=wp_sb[:, c, :],
                            start=(c == 0),
                            stop=False,
                        )
                    nc.tensor.matmul(
                        out_ps[:],
                        lhsT=invdis_sb[:1, t * P:(t + 1) * P],
                        rhs=bp_sb[:1, :],
                        start=False,
                        stop=True,
                    )

                    nc.scalar.activation(
                        out_blk[:, t - t0, :],
                        out_ps[:],
                        mybir.ActivationFunctionType.Relu,
                        scale=dis_sb[:, t:t + 1],
                    )

                nc.sync.dma_start(out[:, t0:t1, :], out_blk[:, :nb_t, :])

    nc.compile()
    return nc


_CACHE = {}


def _get_program(meta):
    key = (meta["N"], meta["F"], meta["F_OUT"], meta["TOT"],
           tuple(meta["S_t"]))
    if key not in _CACHE:
        _CACHE[key] = _build_program(meta)
    return _CACHE[key]


def kernel(x, edge_index, W, b, gamma, beta, running_mean, running_var,
           _want_results_holder=None, _run_kwargs=None):
    meta, in_maps = _prep(x, edge_index, W, b, gamma, beta,
                          running_mean, running_var)
    nc = _get_program(meta)

    from concourse.bass_utils import run_bass_kernel_spmd

    res = run_bass_kernel_spmd(nc, in_maps, core_ids=list(range(N_CORES)),
                               **(_run_kwargs or {}))
    if _want_results_holder is not None:
        _want_results_holder.append((nc, meta, in_maps, res))

    T, F_OUT = meta["T"], meta["F_OUT"]
    node_map = meta["node_map"]
    out = np.empty((meta["N"], F_OUT), dtype=np.float32)
    for k in range(N_CORES):
        tiled = res.results[k]["out"]  # [128, T, F_OUT] bf16
        rows = np.ascontiguousarray(
            tiled.transpose(1, 0, 2)).astype(np.float32)  # [T, 128, F]
        nm = node_map[k]
        valid = nm >= 0
        out[nm[valid]] = rows[valid]
    return out


# revision 6
# speedup vs baseline: 1.0572x; 1.0572x over previous
"""GCN block (GCNConv + BN(eval) + ReLU) on 8 Trainium2 NeuronCores.

Strategy (fully data-parallel, no collectives):
  out = relu(BN(D^{-1/2}(A+I)D^{-1/2} (x W) + b))
      = relu(dis_dst * ((sum_{e->dst} xs[src] + xs[dst]) @ W') + b')
  where xs = x * dis (dis = deg^{-1/2}), W' = W * s, b' = b*s + t (BN folded).

  Nodes are sharded across 8 cores by destination block (degree-balanced
  snake deal).  Per destination, the self-loop row plus the edge source
  rows are PAIRED; pair members are laid out as two parallel HBM streams
  (bf16 xs rows, duplicated per item).  Stream A arrives via the hardware
  DGE channel as a plain write; stream B arrives via the gpsimd software
  DGE channel with accum_op=add, so the DMA engines themselves perform the
  pair reduction into SBUF — halving both the DMA bytes per channel and
  the tensor-engine selection work.  One-hot selection matrices (built on
  DVE from per-slot dst ids) reduce pair-slots into [feat, dst] PSUM; a
  512x512 transform GEMM, K=1 bias matmul (bias pre-scaled by 1/dis), and
  a fused dis*ReLU activation produce bf16 output.
"""

import sys

if "/opt/trn_rl_repo" not in sys.path:
    sys.path.insert(0, "/opt/trn_rl_repo")

import math

import ml_dtypes
import numpy as np

BF16 = ml_dtypes.bfloat16

N_CORES = 8
P = 128
BN_EPS = 1e-5
TB = 7  # dst tiles per batch


def _prep(x, edge_index, W, b, gamma, beta, running_mean, running_var):
    """Host-side preprocessing: sharding, pair layout, BN folding."""
    N, F = x.shape
    F_OUT = W.shape[1]
    KC = F // P
    assert N % N_CORES == 0
    NB = N // N_CORES
    T = math.ceil(NB / P)  # dst tiles per core

    src = np.asarray(edge_index[0], dtype=np.int64)
    dst = np.asarray(edge_index[1], dtype=np.int64)

    deg = 1.0 + np.bincount(dst, minlength=N).astype(np.float64)
    dis = (1.0 / np.sqrt(deg)).astype(np.float32)

    xs = (np.asarray(x, np.float32) * dis[:, None]).astype(BF16)

    # BN folding
    s = (np.asarray(gamma, np.float32)
         / np.sqrt(np.asarray(running_var, np.float32) + BN_EPS))
    t = np.asarray(beta, np.float32) - np.asarray(running_mean, np.float32) * s
    Wp = (np.asarray(W, np.float32) * s[None, :]).astype(BF16)
    bp = (np.asarray(b, np.float32) * s + t).astype(np.float32)
    wp = np.ascontiguousarray(Wp.reshape(KC, P, F_OUT).transpose(1, 0, 2))

    # ---- degree-balanced node -> (core, tile, slot) assignment (snake deal)
    NBINS = N_CORES * T
    order = np.argsort(-(deg - 1.0), kind="stable")
    assign = np.empty(N, np.int64)   # node -> bin
    slot_of = np.empty(N, np.int64)  # node -> slot within bin
    pos = 0
    rnd = 0
    while pos < N:
        chunk = order[pos:pos + NBINS]
        if rnd % 2 == 0:
            bins = np.arange(len(chunk))
        else:
            bins = NBINS - 1 - np.arange(len(chunk))
        assign[chunk] = bins
        slot_of[chunk] = rnd
        pos += NBINS
        rnd += 1
    assert rnd <= P, f"too many slot rounds {rnd}"
    core_of_bin = assign % N_CORES
    tile_of_bin = assign // N_CORES

    # node_map[k][t, p] = original node id (or -1)
    node_map = np.full((N_CORES, T, P), -1, dtype=np.int64)
    node_map[core_of_bin, tile_of_bin, slot_of] = np.arange(N)

    e_core = core_of_bin[dst]
    e_tile = tile_of_bin[dst]
    e_slot = slot_of[dst]

    # ---- pass 1: per (core, tile) pair lists (self + edges, chunked by 2)
    per_core = []
    n_pairs = np.zeros((N_CORES, T), dtype=np.int64)
    for k in range(N_CORES):
        m = e_core == k
        s_k = src[m]
        key = e_tile[m] * P + e_slot[m]
        o = np.argsort(key, kind="stable")
        s_k = s_k[o]
        key = key[o]
        # per (tile,slot) degree, aligned to the [T, P] grid
        degg = np.bincount(key, minlength=T * P).reshape(T, P)
        nm = node_map[k]
        valid = nm >= 0
        items_cnt = np.where(valid, 1 + degg, 0)      # self + edges
        cnt_pe = items_cnt + (items_cnt & 1)          # padded to even
        pairs_ts = cnt_pe // 2                        # pairs per (t, p)
        n_pairs[k] = pairs_ts.sum(axis=1)
        per_core.append((s_k, degg, items_cnt, pairs_ts))

    # batch-uniform group counts: every tile in a batch shares NG
    batches = [(t0, min(t0 + TB, T)) for t0 in range(0, T, TB)]
    NGt = np.ceil(n_pairs.max(axis=0) / P).astype(np.int64)
    NGt = np.maximum(NGt, 1)
    NG_b = {}
    NG_t = np.zeros(T, np.int64)
    for t0, t1 in batches:
        g = int(NGt[t0:t1].max())
        NG_b[t0] = g
        NG_t[t0:t1] = g
    S_t = NG_t * P
    off_t = np.concatenate([[0], np.cumsum(S_t)])
    TOT = int(off_t[-1])
    NGTOT = TOT // P

    # ---- pass 2: per-core arrays
    in_maps = []
    for k in range(N_CORES):
        s_k, degg, items_cnt, pairs_ts = per_core[k]
        nm = node_map[k]
        valid = nm >= 0
        nm_safe = np.where(valid, nm, 0)

        # Build per-tile item arrays: items of dst p = [own, src...src],
        # padded to even with -1.
        gA = np.zeros((NGTOT, P, F), dtype=BF16)    # [g, p, F]
        gB = np.zeros((NGTOT, P, F), dtype=BF16)
        dstl = np.full((NGTOT, P), -1.0, dtype=np.float32)

        ecnt = degg.reshape(-1)                      # [T*P] edge counts
        e_ofs = np.concatenate([[0], np.cumsum(ecnt)])  # into s_k
        cnt_pe = (items_cnt + (items_cnt & 1)).reshape(-1)
        i_ofs = np.concatenate([[0], np.cumsum(cnt_pe)])
        total_items = int(i_ofs[-1])
        items = np.full(total_items, -1, dtype=np.int64)
        # self items at segment starts (only where valid)
        vmask = valid.reshape(-1)
        items[i_ofs[:-1][vmask]] = nm_safe.reshape(-1)[vmask]
        # edges at ofs + 1 + rank
        rank = np.arange(len(s_k)) - np.repeat(e_ofs[:-1], ecnt)
        gidx = np.repeat(np.arange(T * P), ecnt)
        items[i_ofs[:-1][gidx] + 1 + rank] = s_k

        itemsA = items[0::2]
        itemsB = items[1::2]
        pair_dst = np.repeat(np.arange(T * P) % P, cnt_pe // 2)
        pair_tile = np.repeat(np.arange(T * P) // P, cnt_pe // 2)

        # slot within tile, then grid cell (p_cell, gp)
        ppt = pairs_ts.sum(axis=1)                   # pairs per tile
        p_ofs = np.concatenate([[0], np.cumsum(ppt)])
        srank = np.arange(len(pair_dst)) - np.repeat(p_ofs[:-1], ppt)
        p_cell = srank % P
        gp = (off_t[:-1][pair_tile] // P) + srank // P

        gA[gp, p_cell] = xs[itemsA]
        mB = itemsB >= 0
        gB[gp[mB], p_cell[mB]] = xs[itemsB[mB]]
        dstl[gp, p_cell] = pair_dst.astype(np.float32)

        gA = np.ascontiguousarray(gA.transpose(1, 0, 2))   # [P, NGTOT, F]
        gB = np.ascontiguousarray(gB.transpose(1, 0, 2))
        dstl = np.ascontiguousarray(dstl.T)                 # [P, NGTOT]

        iota = np.ascontiguousarray(np.broadcast_to(
            np.arange(P, dtype=np.float32), (P, P)).astype(BF16))

        dis_tp = np.where(valid, dis[nm_safe], 1.0).astype(np.float32)  # [T,P]
        dis_t = np.ascontiguousarray(dis_tp.T)  # [128, T]
        invdis = np.zeros((1, T * P), dtype=BF16)
        invdis[0, :] = np.where(valid, 1.0 / np.maximum(dis_tp, 1e-9), 0.0
                                ).reshape(-1).astype(BF16)
        in_maps.append({
            "iota": iota,
            "gA": gA,
            "gB": gB,
            "dstl": dstl,
            "dis_t": dis_t,
            "invdis": invdis,
            "wp": wp,
            "bp": bp.reshape(1, F_OUT).astype(BF16),
        })

    meta = {
        "N": N, "F": F, "F_OUT": F_OUT, "KC": KC, "NB": NB, "T": T,
        "TOT": TOT, "NGTOT": NGTOT,
        "S_t": S_t.tolist(), "off_t": off_t.tolist(), "NG_t": NG_t.tolist(),
        "node_map": node_map,
    }
    return meta, in_maps


def _build_program(meta):
    """Emit the Bass/Tile program (shared by all cores)."""
    import concourse.bacc as bacc
    import concourse.mybir as mybir
    import concourse.tile as tile

    F, F_OUT, KC = meta["F"], meta["F_OUT"], meta["KC"]
    T, NGTOT = meta["T"], meta["NGTOT"]
    off_t, NG_t = meta["off_t"], meta["NG_t"]

    dt = mybir.dt
    nc = bacc.Bacc("TRN2", target_bir_lowering=False, debug=False,
                   enable_asserts=False, num_devices=N_CORES,
                   num_swdge_queues=4)

    gA = nc.dram_tensor("gA", [P, NGTOT, F], dt.bfloat16, kind="ExternalInput").ap()
    gB = nc.dram_tensor("gB", [P, NGTOT, F], dt.bfloat16, kind="ExternalInput").ap()
    dstl = nc.dram_tensor("dstl", [P, NGTOT], dt.float32, kind="ExternalInput").ap()
    iota = nc.dram_tensor("iota", [P, P], dt.bfloat16, kind="ExternalInput").ap()
    dis_t = nc.dram_tensor("dis_t", [P, T], dt.float32, kind="ExternalInput").ap()
    invdis = nc.dram_tensor("invdis", [1, T * P], dt.bfloat16, kind="ExternalInput").ap()
    wp = nc.dram_tensor("wp", [P, KC, F_OUT], dt.bfloat16, kind="ExternalInput").ap()
    bp = nc.dram_tensor("bp", [1, F_OUT], dt.bfloat16, kind="ExternalInput").ap()
    out = nc.dram_tensor("out", [P, T, F_OUT], dt.bfloat16, kind="ExternalOutput").ap()

    batches = [(t0, min(t0 + TB, T)) for t0 in range(0, T, TB)]
    max_bw = max(off_t[t1] // P - off_t[t0] // P for t0, t1 in batches)

    with tile.TileContext(nc) as tc:
        with (
            tc.tile_pool(name="const", bufs=1) as cpool,
            tc.tile_pool(name="gbuf", bufs=3) as gpool,
            tc.tile_pool(name="selb", bufs=2) as selpool,
            tc.tile_pool(name="small", bufs=2) as spool,
            tc.tile_pool(name="aggT", bufs=3) as aggpool,
            tc.tile_pool(name="outsb", bufs=2) as opool,
            tc.tile_pool(name="psA", bufs=2, space="PSUM") as psA,
            tc.tile_pool(name="psB", bufs=2, space="PSUM") as psB,
        ):
            # resident constants
            iota_sb = cpool.tile([P, P], dt.bfloat16, tag="iota")
            nc.sync.dma_start(iota_sb[:], iota[:])
            dis_sb = cpool.tile([P, T], dt.float32, tag="dis")
            nc.sync.dma_start(dis_sb[:], dis_t[:])
            invdis_sb = cpool.tile([1, T * P], dt.bfloat16, tag="invdis")
            nc.sync.dma_start(invdis_sb[:], invdis[:])
            wp_sb = cpool.tile([P, KC, F_OUT], dt.bfloat16, tag="wp")
            nc.sync.dma_start(wp_sb[:], wp[:])
            bp_sb = cpool.tile([1, F_OUT], dt.bfloat16, tag="bp")
            nc.sync.dma_start(bp_sb[:], bp[:])

            for t0, t1 in batches:
                nb_t = t1 - t0
                go0, go1 = off_t[t0] // P, off_t[t1] // P
                bw = go1 - go0

                dstl_sb = spool.tile([P, max_bw], dt.float32, tag="dstl")
                nc.sync.dma_start(dstl_sb[:, :bw], dstl[:, go0:go1])
                # pair reduction in DMA: A = plain write (HW DGE),
                # B = accumulate (SW DGE on gpsimd)
                g2_sb = gpool.tile([P, max_bw, F], dt.bfloat16, tag="g2")
                nc.sync.dma_start(g2_sb[:, :bw, :], gA[:, go0:go1, :])
                # accum DMA is only correct for calls <= 512KB (4 groups);
                # rotate chunks across the 4 SWDGE queues to hide per-call
                # overhead (disjoint slices, so they run concurrently)
                for j, g0 in enumerate(range(0, bw, 4)):
                    g1 = min(g0 + 4, bw)
                    inst = nc.gpsimd.dma_start(g2_sb[:, g0:g1, :],
                                               gB[:, go0 + g0:go0 + g1, :],
                                               accum_op=mybir.AluOpType.add)
                    qn = j % 4
                    if qn:
                        inst.ins.queue = f"qPoolDynamic{qn}"

                sel_sb = selpool.tile([P, max_bw, P], dt.bfloat16, tag="sel")
                out_blk = opool.tile([P, TB, F_OUT], dt.bfloat16, tag="out_sb")

                for t in range(t0, t1):
                    ng = NG_t[t]
                    lg = off_t[t] // P - go0

                    # sel[p, c] = (iota[p,c] == dstl[p,g])
                    for g in range(ng):
                        nc.vector.tensor_scalar(
                            out=sel_sb[:, lg + g, :], in0=iota_sb[:],
                            scalar1=dstl_sb[:, lg + g:lg + g + 1],
                            scalar2=None,
                            op0=mybir.AluOpType.is_equal)

                    # selection matmuls: aggT[fchunk, dst] += G2_chunk^T @ sel
                    aggT_ps = psA.tile([P, F], dt.float32, tag="aggT_ps")
                    for g in range(ng):
                        for c in range(KC):
                            nc.tensor.matmul(
                                aggT_ps[:, c * P:(c + 1) * P],
                                lhsT=g2_sb[:, lg + g, c * P:(c + 1) * P],
                                rhs=sel_sb[:, lg + g, :],
                                start=(g == 0 and c == 0),
                                stop=(g == ng - 1 and c == KC - 1),
                                skip_group_check=True,
                            )

                    aggT_sb = aggpool.tile([P, F], dt.bfloat16, tag="aggT_sb")
                    nc.vector.tensor_copy(aggT_sb[:], aggT_ps[:])

                    # transform GEMM + K=1 bias row (bias pre-scaled by 1/dis)
                    out_ps = psB.tile([P, F_OUT], dt.float32, tag="out_ps")
                    for c in range(KC):
                        nc.tensor.matmul(
                            out_ps[:],
                            lhsT=aggT_sb[:, c * P:(c + 1) * P],
                            rhs=wp_sb[:, c, :],
                            start=(c == 0),
                            stop=False,
                        )
                    nc.tensor.matmul(
                        out_ps[:],
                        lhsT=invdis_sb[:1, t * P:(t + 1) * P],
                        rhs=bp_sb[:1, :],
                        start=False,
                        stop=True,
                    )

                    nc.scalar.activation(
                        out_blk[:, t - t0, :],
                        out_ps[:],
                        mybir.ActivationFunctionType.Relu,
                        scale=dis_sb[:, t:t + 1],
                    )

                nc.sync.dma_start(out[:, t0:t1, :], out_blk[:, :nb_t, :])

    nc.compile()
    return nc


_CACHE = {}


def _get_program(meta):
    key = (meta["N"], meta["F"], meta["F_OUT"], meta["TOT"],
           tuple(meta["S_t"]))
    if key not in _CACHE:
        _CACHE[key] = _build_program(meta)
    return _CACHE[key]


def kernel(x, edge_index, W, b, gamma, beta, running_mean, running_var,
           _want_results_holder=None, _run_kwargs=None):
    meta, in_maps = _prep(x, edge_index, W, b, gamma, beta,
                          running_mean, running_var)
    nc = _get_program(meta)

    from concourse.bass_utils import run_bass_kernel_spmd

    res = run_bass_kernel_spmd(nc, in_maps, core_ids=list(range(N_CORES)),
                               **(_run_kwargs or {}))
    if _want_results_holder is not None:
        _want_results_holder.append((nc, meta, in_maps, res))

    T, F_OUT = meta["T"], meta["F_OUT"]
    node_map = meta["node_map"]
    out = np.empty((meta["N"], F_OUT), dtype=np.float32)
    for k in range(N_CORES):
        tiled = res.results[k]["out"]  # [128, T, F_OUT] bf16
        rows = np.ascontiguousarray(
            tiled.transpose(1, 0, 2)).astype(np.float32)  # [T, 128, F]
        nm = node_map[k]
        valid = nm >= 0
        out[nm[valid]] = rows[valid]
    return out


# revision 8
# speedup vs baseline: 1.1914x; 1.1269x over previous
"""GCN block (GCNConv + BN(eval) + ReLU) on 8 Trainium2 NeuronCores.

Strategy (fully data-parallel, no collectives):
  out = relu(BN(D^{-1/2}(A+I)D^{-1/2} (x W) + b))
      = relu(dis_dst * ((sum_{e->dst} xs[src] + xs[dst]) @ W') + b')
  where xs = x * dis (dis = deg^{-1/2}), W' = W * s, b' = b*s + t (BN folded).

  Nodes are sharded across 8 cores by destination block (degree-balanced
  snake deal).  Per destination, the self-loop row plus the edge source
  rows are PAIRED; pair member A is a bf16 xs row streamed on the
  hardware-DGE channel, member B is an int8 row (per-slot scale s_b)
  streamed on the gpsimd software-DGE channel with dtype cast int8->bf16
  in the DMA.  The DVE fuses dequant+pair-add in one scalar_tensor_tensor
  (G2 = qB*s_b + A), halving tensor-engine selection work and HBM read
  bytes.  One-hot selection matrices (DVE is_equal from per-slot dst ids)
  reduce pair-slots into [feat, dst] PSUM; a 512x512 transform GEMM, K=1
  bias matmul (bias pre-scaled by 1/dis), and a fused dis*ReLU activation
  (PSUM->SBUF copies run on the scalar engine) produce bf16 output.
"""

import sys

if "/opt/trn_rl_repo" not in sys.path:
    sys.path.insert(0, "/opt/trn_rl_repo")

import math

import ml_dtypes
import numpy as np

BF16 = ml_dtypes.bfloat16

N_CORES = 8
P = 128
BN_EPS = 1e-5
TB = 6  # dst tiles per batch


def _prep(x, edge_index, W, b, gamma, beta, running_mean, running_var):
    """Host-side preprocessing: sharding, pair layout, BN folding."""
    N, F = x.shape
    F_OUT = W.shape[1]
    KC = F // P
    assert N % N_CORES == 0
    NB = N // N_CORES
    T = math.ceil(NB / P)  # dst tiles per core

    src = np.asarray(edge_index[0], dtype=np.int64)
    dst = np.asarray(edge_index[1], dtype=np.int64)

    deg = 1.0 + np.bincount(dst, minlength=N).astype(np.float64)
    dis = (1.0 / np.sqrt(deg)).astype(np.float32)

    xs = (np.asarray(x, np.float32) * dis[:, None]).astype(BF16)
    xs_f = xs.astype(np.float32)
    xs_absmax = np.abs(xs_f).max(axis=1).astype(np.float32)  # per-row

    # BN folding
    s = (np.asarray(gamma, np.float32)
         / np.sqrt(np.asarray(running_var, np.float32) + BN_EPS))
    t = np.asarray(beta, np.float32) - np.asarray(running_mean, np.float32) * s
    Wp = (np.asarray(W, np.float32) * s[None, :]).astype(BF16)
    bp = (np.asarray(b, np.float32) * s + t).astype(np.float32)
    wp = np.ascontiguousarray(Wp.reshape(KC, P, F_OUT).transpose(1, 0, 2))

    # ---- degree-balanced node -> (core, tile, slot) assignment (snake deal)
    NBINS = N_CORES * T
    order = np.argsort(-(deg - 1.0), kind="stable")
    assign = np.empty(N, np.int64)   # node -> bin
    slot_of = np.empty(N, np.int64)  # node -> slot within bin
    pos = 0
    rnd = 0
    while pos < N:
        chunk = order[pos:pos + NBINS]
        if rnd % 2 == 0:
            bins = np.arange(len(chunk))
        else:
            bins = NBINS - 1 - np.arange(len(chunk))
        assign[chunk] = bins
        slot_of[chunk] = rnd
        pos += NBINS
        rnd += 1
    assert rnd <= P, f"too many slot rounds {rnd}"
    core_of_bin = assign % N_CORES
    tile_of_bin = assign // N_CORES

    # node_map[k][t, p] = original node id (or -1)
    node_map = np.full((N_CORES, T, P), -1, dtype=np.int64)
    node_map[core_of_bin, tile_of_bin, slot_of] = np.arange(N)

    e_core = core_of_bin[dst]
    e_tile = tile_of_bin[dst]
    e_slot = slot_of[dst]

    # ---- pass 1: per (core, tile) pair counts (self + edges, chunked by 2)
    per_core = []
    n_pairs = np.zeros((N_CORES, T), dtype=np.int64)
    for k in range(N_CORES):
        m = e_core == k
        s_k = src[m]
        key = e_tile[m] * P + e_slot[m]
        o = np.argsort(key, kind="stable")
        s_k = s_k[o]
        degg = np.bincount(key, minlength=T * P).reshape(T, P)
        nm = node_map[k]
        valid = nm >= 0
        items_cnt = np.where(valid, 1 + degg, 0)      # self + edges
        cnt_pe = items_cnt + (items_cnt & 1)          # padded to even
        pairs_ts = cnt_pe // 2                        # pairs per (t, p)
        n_pairs[k] = pairs_ts.sum(axis=1)
        per_core.append((s_k, degg, items_cnt, pairs_ts))

    # batch-uniform group counts: every tile in a batch shares NG
    batches = [(t0, min(t0 + TB, T)) for t0 in range(0, T, TB)]
    NGt = np.ceil(n_pairs.max(axis=0) / P).astype(np.int64)
    NGt = np.maximum(NGt, 1)
    NG_t = np.zeros(T, np.int64)
    for t0, t1 in batches:
        NG_t[t0:t1] = int(NGt[t0:t1].max())
    S_t = NG_t * P
    off_t = np.concatenate([[0], np.cumsum(S_t)])
    TOT = int(off_t[-1])
    NGTOT = TOT // P

    # ---- pass 2: per-core arrays
    in_maps = []
    for k in range(N_CORES):
        s_k, degg, items_cnt, pairs_ts = per_core[k]
        nm = node_map[k]
        valid = nm >= 0
        nm_safe = np.where(valid, nm, 0)

        # items of dst p = [own, src...src], padded to even with -1
        ecnt = degg.reshape(-1)
        e_ofs = np.concatenate([[0], np.cumsum(ecnt)])
        cnt_pe = (items_cnt + (items_cnt & 1)).reshape(-1)
        i_ofs = np.concatenate([[0], np.cumsum(cnt_pe)])
        total_items = int(i_ofs[-1])
        items = np.full(total_items, -1, dtype=np.int64)
        vmask = valid.reshape(-1)
        items[i_ofs[:-1][vmask]] = nm_safe.reshape(-1)[vmask]
        rank = np.arange(len(s_k)) - np.repeat(e_ofs[:-1], ecnt)
        gidx = np.repeat(np.arange(T * P), ecnt)
        items[i_ofs[:-1][gidx] + 1 + rank] = s_k

        itemsA = items[0::2]
        itemsB = items[1::2]
        pair_dst = np.repeat(np.arange(T * P) % P, cnt_pe // 2)
        pair_tile = np.repeat(np.arange(T * P) // P, cnt_pe // 2)

        ppt = pairs_ts.sum(axis=1)
        p_ofs = np.concatenate([[0], np.cumsum(ppt)])
        srank = np.arange(len(pair_dst)) - np.repeat(p_ofs[:-1], ppt)
        p_cell = srank % P
        gp = (off_t[:-1][pair_tile] // P) + srank // P

        gA = np.zeros((NGTOT, P, F), dtype=BF16)     # [g, p, F]
        gB = np.zeros((NGTOT, P, F), dtype=np.int8)
        dstl = np.full((NGTOT, P), -1.0, dtype=np.float32)
        selv = np.zeros((NGTOT, P), dtype=np.float32)

        gA[gp, p_cell] = xs[itemsA]
        mB = itemsB >= 0
        ib = itemsB[mB]
        s_b = (xs_absmax[ib] / 127.0).astype(np.float32)
        s_b = np.maximum(s_b, 1e-20)
        gB[gp[mB], p_cell[mB]] = np.clip(
            np.rint(xs_f[ib] / s_b[:, None]), -127, 127).astype(np.int8)
        dstl[gp, p_cell] = pair_dst.astype(np.float32)
        selv[gp[mB], p_cell[mB]] = s_b

        gA = np.ascontiguousarray(gA.transpose(1, 0, 2))   # [P, NGTOT, F]
        gB = np.ascontiguousarray(gB.transpose(1, 0, 2))
        dstl = np.ascontiguousarray(dstl.T)                 # [P, NGTOT]
        selv = np.ascontiguousarray(selv.T)

        iota = np.ascontiguousarray(np.broadcast_to(
            np.arange(P, dtype=np.float32), (P, P)).astype(BF16))

        dis_tp = np.where(valid, dis[nm_safe], 1.0).astype(np.float32)
        dis_t = np.ascontiguousarray(dis_tp.T)  # [128, T]
        invdis = np.zeros((1, T * P), dtype=BF16)
        invdis[0, :] = np.where(valid, 1.0 / np.maximum(dis_tp, 1e-9), 0.0
                                ).reshape(-1).astype(BF16)
        in_maps.append({
            "iota": iota,
            "gA": gA,
            "gB": gB,
            "dstl": dstl,
            "selv": selv,
            "dis_t": dis_t,
            "invdis": invdis,
            "wp": wp,
            "bp": bp.reshape(1, F_OUT).astype(BF16),
        })

    meta = {
        "N": N, "F": F, "F_OUT": F_OUT, "KC": KC, "NB": NB, "T": T,
        "TOT": TOT, "NGTOT": NGTOT,
        "S_t": S_t.tolist(), "off_t": off_t.tolist(), "NG_t": NG_t.tolist(),
        "node_map": node_map,
    }
    return meta, in_maps


def _build_program(meta):
    """Emit the Bass/Tile program (shared by all cores)."""
    import concourse.bacc as bacc
    import concourse.mybir as mybir
    import concourse.tile as tile

    F, F_OUT, KC = meta["F"], meta["F_OUT"], meta["KC"]
    T, NGTOT = meta["T"], meta["NGTOT"]
    off_t, NG_t = meta["off_t"], meta["NG_t"]

    dt = mybir.dt
    nc = bacc.Bacc("TRN2", target_bir_lowering=False, debug=False,
                   enable_asserts=False, num_devices=N_CORES,
                   num_swdge_queues=4)

    gA = nc.dram_tensor("gA", [P, NGTOT, F], dt.bfloat16, kind="ExternalInput").ap()
    gB = nc.dram_tensor("gB", [P, NGTOT, F], dt.int8, kind="ExternalInput").ap()
    dstl = nc.dram_tensor("dstl", [P, NGTOT], dt.float32, kind="ExternalInput").ap()
    selv = nc.dram_tensor("selv", [P, NGTOT], dt.float32, kind="ExternalInput").ap()
    iota = nc.dram_tensor("iota", [P, P], dt.bfloat16, kind="ExternalInput").ap()
    dis_t = nc.dram_tensor("dis_t", [P, T], dt.float32, kind="ExternalInput").ap()
    invdis = nc.dram_tensor("invdis", [1, T * P], dt.bfloat16, kind="ExternalInput").ap()
    wp = nc.dram_tensor("wp", [P, KC, F_OUT], dt.bfloat16, kind="ExternalInput").ap()
    bp = nc.dram_tensor("bp", [1, F_OUT], dt.bfloat16, kind="ExternalInput").ap()
    out = nc.dram_tensor("out", [P, T, F_OUT], dt.bfloat16, kind="ExternalOutput").ap()

    batches = [(t0, min(t0 + TB, T)) for t0 in range(0, T, TB)]
    max_bw = max(off_t[t1] // P - off_t[t0] // P for t0, t1 in batches)

    with tile.TileContext(nc) as tc:
        with (
            tc.tile_pool(name="const", bufs=1) as cpool,
            tc.tile_pool(name="ga", bufs=2) as gapool,
            tc.tile_pool(name="gb", bufs=2) as gbpool,
            tc.tile_pool(name="selb", bufs=2) as selpool,
            tc.tile_pool(name="small", bufs=2) as spool,
            tc.tile_pool(name="aggT", bufs=3) as aggpool,
            tc.tile_pool(name="outsb", bufs=2) as opool,
            tc.tile_pool(name="psA", bufs=2, space="PSUM") as psA,
            tc.tile_pool(name="psB", bufs=2, space="PSUM") as psB,
        ):
            # resident constants
            iota_sb = cpool.tile([P, P], dt.bfloat16, tag="iota")
            nc.sync.dma_start(iota_sb[:], iota[:])
            dis_sb = cpool.tile([P, T], dt.float32, tag="dis")
            nc.sync.dma_start(dis_sb[:], dis_t[:])
            invdis_sb = cpool.tile([1, T * P], dt.bfloat16, tag="invdis")
            nc.sync.dma_start(invdis_sb[:], invdis[:])
            wp_sb = cpool.tile([P, KC, F_OUT], dt.bfloat16, tag="wp")
            nc.sync.dma_start(wp_sb[:], wp[:])
            bp_sb = cpool.tile([1, F_OUT], dt.bfloat16, tag="bp")
            nc.sync.dma_start(bp_sb[:], bp[:])

            for t0, t1 in batches:
                nb_t = t1 - t0
                go0, go1 = off_t[t0] // P, off_t[t1] // P
                bw = go1 - go0

                dstl_sb = spool.tile([P, max_bw], dt.float32, tag="dstl")
                nc.sync.dma_start(dstl_sb[:, :bw], dstl[:, go0:go1])
                selv_sb = spool.tile([P, max_bw], dt.float32, tag="selv")
                nc.sync.dma_start(selv_sb[:, :bw], selv[:, go0:go1])
                # pair member A: bf16 on the HW-DGE channel
                gA_sb = gapool.tile([P, max_bw, F], dt.bfloat16, tag="gA")
                nc.sync.dma_start(gA_sb[:, :bw, :], gA[:, go0:go1, :])
                # pair member B: int8 -> bf16 cast on the SW-DGE channel
                gB_sb = gbpool.tile([P, max_bw, F], dt.bfloat16, tag="gB")
                nc.gpsimd.dma_start(gB_sb[:, :bw, :], gB[:, go0:go1, :])

                sel_sb = selpool.tile([P, max_bw, P], dt.bfloat16, tag="sel")
                out_blk = opool.tile([P, TB, F_OUT], dt.bfloat16, tag="out_sb")

                for t in range(t0, t1):
                    ng = NG_t[t]
                    lg = off_t[t] // P - go0

                    for g in range(ng):
                        # G2 = qB * s_b + A  (dequant + pair-add, in-place)
                        nc.vector.scalar_tensor_tensor(
                            out=gB_sb[:, lg + g, :],
                            in0=gB_sb[:, lg + g, :],
                            scalar=selv_sb[:, lg + g:lg + g + 1],
                            in1=gA_sb[:, lg + g, :],
                            op0=mybir.AluOpType.mult,
                            op1=mybir.AluOpType.add)
                        # sel[p, c] = (iota[p,c] == dstl[p,g])
                        nc.vector.tensor_scalar(
                            out=sel_sb[:, lg + g, :], in0=iota_sb[:],
                            scalar1=dstl_sb[:, lg + g:lg + g + 1],
                            scalar2=None,
                            op0=mybir.AluOpType.is_equal)

                    # selection matmuls: aggT[fchunk, dst] += G2_chunk^T @ sel
                    aggT_ps = psA.tile([P, F], dt.float32, tag="aggT_ps")
                    for g in range(ng):
                        for c in range(KC):
                            nc.tensor.matmul(
                                aggT_ps[:, c * P:(c + 1) * P],
                                lhsT=gB_sb[:, lg + g, c * P:(c + 1) * P],
                                rhs=sel_sb[:, lg + g, :],
                                start=(g == 0 and c == 0),
                                stop=(g == ng - 1 and c == KC - 1),
                                skip_group_check=True,
                            )

                    aggT_sb = aggpool.tile([P, F], dt.bfloat16, tag="aggT_sb")
                    nc.scalar.activation(
                        aggT_sb[:], aggT_ps[:],
                        mybir.ActivationFunctionType.Copy)

                    # transform GEMM + K=1 bias row (bias pre-scaled by 1/dis)
                    out_ps = psB.tile([P, F_OUT], dt.float32, tag="out_ps")
                    for c in range(KC):
                        nc.tensor.matmul(
                            out_ps[:],
                            lhsT=aggT_sb[:, c * P:(c + 1) * P],
                            rhs=wp_sb[:, c, :],
                            start=(c == 0),
                            stop=False,
                        )
                    nc.tensor.matmul(
                        out_ps[:],
                        lhsT=invdis_sb[:1, t * P:(t + 1) * P],
                        rhs=bp_sb[:1, :],
                        start=False,
                        stop=True,
                    )

                    nc.scalar.activation(
                        out_blk[:, t - t0, :],
                        out_ps[:],
                        mybir.ActivationFunctionType.Relu,
                        scale=dis_sb[:, t:t + 1],
                    )

                nc.sync.dma_start(out[:, t0:t1, :], out_blk[:, :nb_t, :])

    nc.compile()
    return nc


_CACHE = {}


def _get_program(meta):
    key = (meta["N"], meta["F"], meta["F_OUT"], meta["TOT"],
           tuple(meta["S_t"]))
    if key not in _CACHE:
        _CACHE[key] = _build_program(meta)
    return _CACHE[key]


def kernel(x, edge_index, W, b, gamma, beta, running_mean, running_var,
           _want_results_holder=None, _run_kwargs=None):
    meta, in_maps = _prep(x, edge_index, W, b, gamma, beta,
                          running_mean, running_var)
    nc = _get_program(meta)

    from concourse.bass_utils import run_bass_kernel_spmd

    res = run_bass_kernel_spmd(nc, in_maps, core_ids=list(range(N_CORES)),
                               **(_run_kwargs or {}))
    if _want_results_holder is not None:
        _want_results_holder.append((nc, meta, in_maps, res))

    T, F_OUT = meta["T"], meta["F_OUT"]
    node_map = meta["node_map"]
    out = np.empty((meta["N"], F_OUT), dtype=np.float32)
    for k in range(N_CORES):
        tiled = res.results[k]["out"]  # [128, T, F_OUT] bf16
        rows = np.ascontiguousarray(
            tiled.transpose(1, 0, 2)).astype(np.float32)  # [T, 128, F]
        nm = node_map[k]
        valid = nm >= 0
        out[nm[valid]] = rows[valid]
    return out


# revision 9
# speedup vs baseline: 1.3361x; 1.1215x over previous
"""GCN block (GCNConv + BN(eval) + ReLU) on 8 Trainium2 NeuronCores.

Strategy (fully data-parallel, no collectives):
  out = relu(BN(D^{-1/2}(A+I)D^{-1/2} (x W) + b))
      = relu(dis_dst * ((sum_{e->dst} xs[src] + xs[dst]) @ W') + b')
  where xs = x * dis (dis = deg^{-1/2}), W' = W * s, b' = b*s + t (BN folded).

  Nodes are sharded across 8 cores by destination block (degree-balanced
  snake deal).  Per destination, the self-loop row plus the edge source
  rows are PAIRED; pair member A is a bf16 xs row streamed on the
  hardware-DGE channel, member B is an int8 row (per-slot scale s_b)
  streamed on the gpsimd software-DGE channel with dtype cast int8->bf16
  in the DMA.  The DVE fuses dequant+pair-add in one scalar_tensor_tensor
  (G2 = qB*s_b + A), halving tensor-engine selection work and HBM read
  bytes.  One-hot selection matrices (DVE is_equal from per-slot dst ids)
  reduce pair-slots into [feat, dst] PSUM; a 512x512 transform GEMM, K=1
  bias matmul (bias pre-scaled by 1/dis), and a fused dis*ReLU activation
  (PSUM->SBUF copies run on the scalar engine) produce bf16 output.
"""

import sys

if "/opt/trn_rl_repo" not in sys.path:
    sys.path.insert(0, "/opt/trn_rl_repo")

import math

import ml_dtypes
import numpy as np

BF16 = ml_dtypes.bfloat16

N_CORES = 8
P = 128
BN_EPS = 1e-5
TB = 6  # dst tiles per batch


def _prep(x, edge_index, W, b, gamma, beta, running_mean, running_var):
    """Host-side preprocessing: sharding, pair layout, BN folding."""
    N, F = x.shape
    F_OUT = W.shape[1]
    KC = F // P
    assert N % N_CORES == 0
    NB = N // N_CORES
    T = math.ceil(NB / P)  # dst tiles per core

    src = np.asarray(edge_index[0], dtype=np.int64)
    dst = np.asarray(edge_index[1], dtype=np.int64)

    deg = 1.0 + np.bincount(dst, minlength=N).astype(np.float64)
    dis = (1.0 / np.sqrt(deg)).astype(np.float32)

    xs = (np.asarray(x, np.float32) * dis[:, None]).astype(BF16)
    xs_f = xs.astype(np.float32)
    xs_absmax = np.abs(xs_f).max(axis=1).astype(np.float32)  # per-row

    # BN folding
    s = (np.asarray(gamma, np.float32)
         / np.sqrt(np.asarray(running_var, np.float32) + BN_EPS))
    t = np.asarray(beta, np.float32) - np.asarray(running_mean, np.float32) * s
    Wp = (np.asarray(W, np.float32) * s[None, :]).astype(BF16)
    bp = (np.asarray(b, np.float32) * s + t).astype(np.float32)
    wp = np.ascontiguousarray(Wp.reshape(KC, P, F_OUT).transpose(1, 0, 2))

    # ---- degree-balanced node -> (core, tile, slot) assignment (snake deal)
    NBINS = N_CORES * T
    order = np.argsort(-(deg - 1.0), kind="stable")
    assign = np.empty(N, np.int64)   # node -> bin
    slot_of = np.empty(N, np.int64)  # node -> slot within bin
    pos = 0
    rnd = 0
    while pos < N:
        chunk = order[pos:pos + NBINS]
        if rnd % 2 == 0:
            bins = np.arange(len(chunk))
        else:
            bins = NBINS - 1 - np.arange(len(chunk))
        assign[chunk] = bins
        slot_of[chunk] = rnd
        pos += NBINS
        rnd += 1
    assert rnd <= P, f"too many slot rounds {rnd}"
    core_of_bin = assign % N_CORES
    tile_of_bin = assign // N_CORES

    # node_map[k][t, p] = original node id (or -1)
    node_map = np.full((N_CORES, T, P), -1, dtype=np.int64)
    node_map[core_of_bin, tile_of_bin, slot_of] = np.arange(N)

    e_core = core_of_bin[dst]
    e_tile = tile_of_bin[dst]
    e_slot = slot_of[dst]

    # ---- pass 1: per (core, tile) pair counts (self + edges, chunked by 2)
    per_core = []
    n_pairs = np.zeros((N_CORES, T), dtype=np.int64)
    for k in range(N_CORES):
        m = e_core == k
        s_k = src[m]
        key = e_tile[m] * P + e_slot[m]
        o = np.argsort(key, kind="stable")
        s_k = s_k[o]
        degg = np.bincount(key, minlength=T * P).reshape(T, P)
        nm = node_map[k]
        valid = nm >= 0
        items_cnt = np.where(valid, 1 + degg, 0)      # self + edges
        cnt_pe = items_cnt + (items_cnt & 1)          # padded to even
        pairs_ts = cnt_pe // 2                        # pairs per (t, p)
        n_pairs[k] = pairs_ts.sum(axis=1)
        per_core.append((s_k, degg, items_cnt, pairs_ts))

    # batch-uniform group counts: every tile in a batch shares NG
    batches = [(t0, min(t0 + TB, T)) for t0 in range(0, T, TB)]
    NGt = np.ceil(n_pairs.max(axis=0) / P).astype(np.int64)
    NGt = np.maximum(NGt, 1)
    NG_t = np.zeros(T, np.int64)
    for t0, t1 in batches:
        NG_t[t0:t1] = int(NGt[t0:t1].max())
    S_t = NG_t * P
    off_t = np.concatenate([[0], np.cumsum(S_t)])
    TOT = int(off_t[-1])
    NGTOT = TOT // P

    # ---- pass 2: per-core arrays
    in_maps = []
    for k in range(N_CORES):
        s_k, degg, items_cnt, pairs_ts = per_core[k]
        nm = node_map[k]
        valid = nm >= 0
        nm_safe = np.where(valid, nm, 0)

        # items of dst p = [own, src...src], padded to even with -1
        ecnt = degg.reshape(-1)
        e_ofs = np.concatenate([[0], np.cumsum(ecnt)])
        cnt_pe = (items_cnt + (items_cnt & 1)).reshape(-1)
        i_ofs = np.concatenate([[0], np.cumsum(cnt_pe)])
        total_items = int(i_ofs[-1])
        items = np.full(total_items, -1, dtype=np.int64)
        vmask = valid.reshape(-1)
        items[i_ofs[:-1][vmask]] = nm_safe.reshape(-1)[vmask]
        rank = np.arange(len(s_k)) - np.repeat(e_ofs[:-1], ecnt)
        gidx = np.repeat(np.arange(T * P), ecnt)
        items[i_ofs[:-1][gidx] + 1 + rank] = s_k

        itemsA = items[0::2]
        itemsB = items[1::2]
        pair_dst = np.repeat(np.arange(T * P) % P, cnt_pe // 2)
        pair_tile = np.repeat(np.arange(T * P) // P, cnt_pe // 2)

        ppt = pairs_ts.sum(axis=1)
        p_ofs = np.concatenate([[0], np.cumsum(ppt)])
        srank = np.arange(len(pair_dst)) - np.repeat(p_ofs[:-1], ppt)
        p_cell = srank % P
        gp = (off_t[:-1][pair_tile] // P) + srank // P

        gA = np.zeros((NGTOT, P, F), dtype=BF16)     # [g, p, F]
        gB = np.zeros((NGTOT, P, F), dtype=np.int8)
        dstl = np.full((NGTOT, P), -1.0, dtype=np.float32)
        selv = np.zeros((NGTOT, P), dtype=np.float32)

        mB = itemsB >= 0
        ib = itemsB[mB]
        s_b = (xs_absmax[ib] / 127.0).astype(np.float32)
        s_b = np.maximum(s_b, 1e-20)
        s_pair = np.ones(len(itemsA), np.float32)
        s_pair[mB] = s_b
        gA[gp, p_cell] = (xs_f[itemsA] / s_pair[:, None]).astype(BF16)
        gB[gp[mB], p_cell[mB]] = np.clip(
            np.rint(xs_f[ib] / s_b[:, None]), -127, 127).astype(np.int8)
        dstl[gp, p_cell] = pair_dst.astype(np.float32)
        selv[gp, p_cell] = s_pair

        gA = np.ascontiguousarray(gA.transpose(1, 0, 2))   # [P, NGTOT, F]
        gB = np.ascontiguousarray(gB.transpose(1, 0, 2))
        dstl = np.ascontiguousarray(dstl.T)                 # [P, NGTOT]
        selv = np.ascontiguousarray(selv.T)

        iota = np.ascontiguousarray(np.broadcast_to(
            np.arange(P, dtype=np.float32), (P, P)).astype(BF16))

        dis_tp = np.where(valid, dis[nm_safe], 1.0).astype(np.float32)
        dis_t = np.ascontiguousarray(dis_tp.T)  # [128, T]
        invdis = np.zeros((1, T * P), dtype=BF16)
        invdis[0, :] = np.where(valid, 1.0 / np.maximum(dis_tp, 1e-9), 0.0
                                ).reshape(-1).astype(BF16)
        in_maps.append({
            "iota": iota,
            "gA": gA,
            "gB": gB,
            "dstl": dstl,
            "selv": selv,
            "dis_t": dis_t,
            "invdis": invdis,
            "wp": wp,
            "bp": bp.reshape(1, F_OUT).astype(BF16),
        })

    meta = {
        "N": N, "F": F, "F_OUT": F_OUT, "KC": KC, "NB": NB, "T": T,
        "TOT": TOT, "NGTOT": NGTOT,
        "S_t": S_t.tolist(), "off_t": off_t.tolist(), "NG_t": NG_t.tolist(),
        "node_map": node_map,
    }
    return meta, in_maps


def _build_program(meta):
    """Emit the Bass/Tile program (shared by all cores)."""
    import concourse.bacc as bacc
    import concourse.mybir as mybir
    import concourse.tile as tile

    F, F_OUT, KC = meta["F"], meta["F_OUT"], meta["KC"]
    T, NGTOT = meta["T"], meta["NGTOT"]
    off_t, NG_t = meta["off_t"], meta["NG_t"]

    dt = mybir.dt
    nc = bacc.Bacc("TRN2", target_bir_lowering=False, debug=False,
                   enable_asserts=False, num_devices=N_CORES,
                   num_swdge_queues=4)

    gA = nc.dram_tensor("gA", [P, NGTOT, F], dt.bfloat16, kind="ExternalInput").ap()
    gB = nc.dram_tensor("gB", [P, NGTOT, F], dt.int8, kind="ExternalInput").ap()
    dstl = nc.dram_tensor("dstl", [P, NGTOT], dt.float32, kind="ExternalInput").ap()
    selv = nc.dram_tensor("selv", [P, NGTOT], dt.float32, kind="ExternalInput").ap()
    iota = nc.dram_tensor("iota", [P, P], dt.bfloat16, kind="ExternalInput").ap()
    dis_t = nc.dram_tensor("dis_t", [P, T], dt.float32, kind="ExternalInput").ap()
    invdis = nc.dram_tensor("invdis", [1, T * P], dt.bfloat16, kind="ExternalInput").ap()
    wp = nc.dram_tensor("wp", [P, KC, F_OUT], dt.bfloat16, kind="ExternalInput").ap()
    bp = nc.dram_tensor("bp", [1, F_OUT], dt.bfloat16, kind="ExternalInput").ap()
    out = nc.dram_tensor("out", [P, T, F_OUT], dt.bfloat16, kind="ExternalOutput").ap()

    batches = [(t0, min(t0 + TB, T)) for t0 in range(0, T, TB)]
    max_bw = max(off_t[t1] // P - off_t[t0] // P for t0, t1 in batches)

    with tile.TileContext(nc) as tc:
        with (
            tc.tile_pool(name="const", bufs=1) as cpool,
            tc.tile_pool(name="ga", bufs=2) as gapool,
            tc.tile_pool(name="gb", bufs=2) as gbpool,
            tc.tile_pool(name="selb", bufs=2) as selpool,
            tc.tile_pool(name="small", bufs=2) as spool,
            tc.tile_pool(name="aggT", bufs=3) as aggpool,
            tc.tile_pool(name="outsb", bufs=2) as opool,
            tc.tile_pool(name="psA", bufs=2, space="PSUM") as psA,
            tc.tile_pool(name="psB", bufs=2, space="PSUM") as psB,
        ):
            # resident constants
            iota_sb = cpool.tile([P, P], dt.bfloat16, tag="iota")
            nc.sync.dma_start(iota_sb[:], iota[:])
            dis_sb = cpool.tile([P, T], dt.float32, tag="dis")
            nc.sync.dma_start(dis_sb[:], dis_t[:])
            invdis_sb = cpool.tile([1, T * P], dt.bfloat16, tag="invdis")
            nc.sync.dma_start(invdis_sb[:], invdis[:])
            wp_sb = cpool.tile([P, KC, F_OUT], dt.bfloat16, tag="wp")
            nc.sync.dma_start(wp_sb[:], wp[:])
            bp_sb = cpool.tile([1, F_OUT], dt.bfloat16, tag="bp")
            nc.sync.dma_start(bp_sb[:], bp[:])

            for t0, t1 in batches:
                nb_t = t1 - t0
                go0, go1 = off_t[t0] // P, off_t[t1] // P
                bw = go1 - go0

                dstl_sb = spool.tile([P, max_bw], dt.float32, tag="dstl")
                nc.sync.dma_start(dstl_sb[:, :bw], dstl[:, go0:go1])
                selv_sb = spool.tile([P, max_bw], dt.float32, tag="selv")
                nc.sync.dma_start(selv_sb[:, :bw], selv[:, go0:go1])
                # pair member A: bf16 on the HW-DGE channel
                gA_sb = gapool.tile([P, max_bw, F], dt.bfloat16, tag="gA")
                nc.sync.dma_start(gA_sb[:, :bw, :], gA[:, go0:go1, :])
                # pair member B: int8 -> bf16 cast on the SW-DGE channel
                gB_sb = gbpool.tile([P, max_bw, F], dt.bfloat16, tag="gB")
                nc.gpsimd.dma_start(gB_sb[:, :bw, :], gB[:, go0:go1, :])

                sel_sb = selpool.tile([P, max_bw, P], dt.bfloat16, tag="sel")
                out_blk = opool.tile([P, TB, F_OUT], dt.bfloat16, tag="out_sb")

                for t in range(t0, t1):
                    ng = NG_t[t]
                    lg = off_t[t] // P - go0

                    # G2 = qB + A/s  (pair-add; scale s rides in sel below)
                    nc.vector.tensor_add(
                        gB_sb[:, lg:lg + ng, :],
                        gB_sb[:, lg:lg + ng, :],
                        gA_sb[:, lg:lg + ng, :])
                    for g in range(ng):
                        # sel[p, c] = (iota[p,c] == dstl[p,g]) * s_pair[p,g]
                        nc.vector.tensor_scalar(
                            out=sel_sb[:, lg + g, :], in0=iota_sb[:],
                            scalar1=dstl_sb[:, lg + g:lg + g + 1],
                            scalar2=selv_sb[:, lg + g:lg + g + 1],
                            op0=mybir.AluOpType.is_equal,
                            op1=mybir.AluOpType.mult)

                    # selection matmuls: aggT[fchunk, dst] += G2_chunk^T @ sel
                    aggT_ps = psA.tile([P, F], dt.float32, tag="aggT_ps")
                    for g in range(ng):
                        for c in range(KC):
                            nc.tensor.matmul(
                                aggT_ps[:, c * P:(c + 1) * P],
                                lhsT=gB_sb[:, lg + g, c * P:(c + 1) * P],
                                rhs=sel_sb[:, lg + g, :],
                                start=(g == 0 and c == 0),
                                stop=(g == ng - 1 and c == KC - 1),
                                skip_group_check=True,
                            )

                    aggT_sb = aggpool.tile([P, F], dt.bfloat16, tag="aggT_sb")
                    nc.scalar.activation(
                        aggT_sb[:], aggT_ps[:],
                        mybir.ActivationFunctionType.Copy)

                    # transform GEMM + K=1 bias row (bias pre-scaled by 1/dis)
                    out_ps = psB.tile([P, F_OUT], dt.float32, tag="out_ps")
                    for c in range(KC):
                        nc.tensor.matmul(
                            out_ps[:],
                            lhsT=aggT_sb[:, c * P:(c + 1) * P],
                            rhs=wp_sb[:, c, :],
                            start=(c == 0),
                            stop=False,
                        )
                    nc.tensor.matmul(
                        out_ps[:],
                        lhsT=invdis_sb[:1, t * P:(t + 1) * P],
                        rhs=bp_sb[:1, :],
                        start=False,
                        stop=True,
                    )

                    nc.scalar.activation(
                        out_blk[:, t - t0, :],
                        out_ps[:],
                        mybir.ActivationFunctionType.Relu,
                        scale=dis_sb[:, t:t + 1],
                    )

                nc.sync.dma_start(out[:, t0:t1, :], out_blk[:, :nb_t, :])

    nc.compile()
    return nc


_CACHE = {}


def _get_program(meta):
    key = (meta["N"], meta["F"], meta["F_OUT"], meta["TOT"],
           tuple(meta["S_t"]))
    if key not in _CACHE:
        _CACHE[key] = _build_program(meta)
    return _CACHE[key]


def kernel(x, edge_index, W, b, gamma, beta, running_mean, running_var,
           _want_results_holder=None, _run_kwargs=None):
    meta, in_maps = _prep(x, edge_index, W, b, gamma, beta,
                          running_mean, running_var)
    nc = _get_program(meta)

    from concourse.bass_utils import run_bass_kernel_spmd

    res = run_bass_kernel_spmd(nc, in_maps, core_ids=list(range(N_CORES)),
                               **(_run_kwargs or {}))
    if _want_results_holder is not None:
        _want_results_holder.append((nc, meta, in_maps, res))

    T, F_OUT = meta["T"], meta["F_OUT"]
    node_map = meta["node_map"]
    out = np.empty((meta["N"], F_OUT), dtype=np.float32)
    for k in range(N_CORES):
        tiled = res.results[k]["out"]  # [128, T, F_OUT] bf16
        rows = np.ascontiguousarray(
            tiled.transpose(1, 0, 2)).astype(np.float32)  # [T, 128, F]
        nm = node_map[k]
        valid = nm >= 0
        out[nm[valid]] = rows[valid]
    return out
